# revision 12
# baseline (speedup 1.0000x reference)
"""Trainium2 Bass kernel for nn_GCNLearnableModel (3-type heterograph GCN, 9 relations,
3 layers) on 8 NeuronCores.

v2 strategy (graph/data parallel, one SPMD NEFF):
 - Nodes of each type sharded 8 ways (6250/core, padded 6272). Each core owns the
   incoming edges of its dst shard.
 - Per layer: ONE AllGather per node type of the feature-major hidden state
   (bounce [64,6272] bf16 -> h_all [8,64,6272]); each core then builds ALL
   relation message tables locally (T_r = onorm_r * (h[src] @ W_lr), bf16,
   node-major, padded-global row ids). Layer-1 tables are folded straight from
   the input features (A_r = embW @ W_1r precomputed on host) -- no embed pass,
   no CC before layer 1. emb_b's contribution is a host-precomputed rank-1
   term added to acc.
 - Gather-aggregate per (layer, relation): edges sorted by (dst window, src
   parity) so each 128-edge block is parity-homogeneous: dma_gather pair-rows
   (256B) with int16 idx = g>>1, one fp8 one-hot S matmul per block
   (rhs = even or odd half of the gathered pair), PSUM-accumulated per window,
   scaled by in_norm into SBUF acc.
 - LayerNorm+ReLU in place on acc; per-type classifier at the end.
"""
import numpy as np
import ml_dtypes

import concourse.bass as bass
import concourse.bacc as bacc
import concourse.mybir as mybir
import concourse.tile as tile
from concourse.bass_utils import run_bass_kernel_spmd
from concourse.masks import make_identity


def _make_runner(nc, n_cores):
    """jit-once PJRT runner with reusable device inputs (mirrors run_bass_via_pjrt)."""
    import jax
    from jax.sharding import Mesh, PartitionSpec
    from jax.experimental.shard_map import shard_map
    from concourse import bass2jax, mybir as mb

    bass2jax.install_neuronx_cc_hook()
    in_names, out_names, out_avals = [], [], []
    pname = nc.partition_id_tensor.name if nc.partition_id_tensor else None
    for alloc in nc.m.functions[0].allocations:
        if not isinstance(alloc, mb.MemoryLocationSet):
            continue
        name = alloc.memorylocations[0].name
        if alloc.kind == "ExternalInput":
            if name != pname:
                in_names.append(name)
        elif alloc.kind == "ExternalOutput":
            out_names.append(name)
            out_avals.append(jax.core.ShapedArray(tuple(alloc.tensor_shape),
                                                  mb.dt.np(alloc.dtype)))
    n_params = len(in_names)
    all_names = in_names + out_names + ([pname] if pname else [])

    def _body(*args):
        operands = list(args)
        if pname:
            operands.append(bass2jax.partition_id_tensor())
        return tuple(bass2jax._bass_exec_p.bind(
            *operands, out_avals=tuple(out_avals), in_names=tuple(all_names),
            out_names=tuple(out_names), lowering_input_output_aliases=(),
            sim_require_finite=True, sim_require_nnan=True, nc=nc))

    devices = jax.devices()[:n_cores]
    mesh = Mesh(np.asarray(devices), ("core",))
    nin = n_params + len(out_names)
    fn = jax.jit(
        shard_map(_body, mesh=mesh, in_specs=(PartitionSpec("core"),) * nin,
                  out_specs=(PartitionSpec("core"),) * len(out_names),
                  check_rep=False),
        keep_unused=True)
    sharding = jax.sharding.NamedSharding(mesh, PartitionSpec("core"))
    return fn, in_names, out_names, out_avals, sharding

# problem constants (hardcoded per harness contract)
REL = [(0, 1), (2, 1), (2, 0), (0, 0), (1, 2), (1, 0), (0, 0), (1, 1), (2, 2)]
N, IN_F, EMB, HID, OUT, NREL, NLAYERS, E = 50000, 128, 64, 64, 8, 9, 3, 800000
EPS = 1e-5
NC = 8                     # cores
SH = N // NC               # shard size 6250
NW = 49                    # windows of 128 nodes per shard (49*128 = 6272)
SHP = NW * 128             # padded shard 6272
NP_ = SHP * NC             # padded global rows 50176
NB = NP_ // 128            # global 128-blocks 392
CHUNK = 1024               # edge slots per dma_gather call (SWDGE ring holds 1024 descs)
BPC = CHUNK // 128         # blocks per chunk (32)

F32, BF16, FP8, I16 = mybir.dt.float32, mybir.dt.bfloat16, mybir.dt.float8e4, mybir.dt.int16
NP_BF16 = np.dtype(mybir.dt.np(BF16))
NP_FP8 = np.dtype(mybir.dt.np(FP8))

RELS_OF_T = [[r for r in range(NREL) if REL[r][1] == t] for t in range(3)]
RELS_OF_S = [[r for r in range(NREL) if REL[r][0] == s] for s in range(3)]
T_ORDER = {l: [(l - 1 + i) % 3 for i in range(3)] for l in (1, 2, 3)}
# order in which src-type tables become available for layer l
BUILD_ORDER = {1: [0, 1, 2], 2: T_ORDER[1], 3: T_ORDER[2]}


# --------------------------------------------------------------------------- host plan
def _plan(src, dst):
    """Edge plan: parity-homogeneous 128-edge blocks per dst window.

    Returns onorm, inorm [NREL,N]; blocks_r[r] = list of (window, parity,
    is_first, is_last) shared across cores; nchunk[r]; per_core[d][r] =
    (idx_rep [128, ns/16] i16, s_arr [128, ns] fp8); coef [NC, NREL, SH]
    (emb_b rank-1 correction, layer 1 only).
    """
    onorm = np.empty((NREL, N), np.float32)
    inorm = np.empty((NREL, N), np.float32)
    for r in range(NREL):
        od = np.bincount(src[r], minlength=N).astype(np.float32)
        idg = np.bincount(dst[r], minlength=N).astype(np.float32)
        onorm[r] = np.maximum(od, 1.0) ** -0.5
        inorm[r] = np.maximum(idg, 1.0) ** -0.5

    per_core = [dict() for _ in range(NC)]
    blocks_r = {}
    nchunk = {}
    coef = np.zeros((NC, NREL, SH), np.float32)
    for r in range(NREL):
        g = (src[r] // SH) * SHP + (src[r] % SH)      # padded global id
        par = g & 1
        dcore = dst[r] // SH
        dloc = dst[r] - dcore * SH
        w = dloc >> 7
        dmod = dloc & 127

        osrc = onorm[r][src[r]]
        dsum = np.bincount(dst[r], weights=osrc, minlength=N).astype(np.float32)
        for d in range(NC):
            sl = slice(d * SH, (d + 1) * SH)
            coef[d, r] = inorm[r, sl] * dsum[sl]

        cnt = np.zeros((NC, NW, 2), np.int64)
        keys = []
        for d in range(NC):
            m = dcore == d
            ww, pp, gg, dm = w[m], par[m], g[m], dmod[m]
            order = np.lexsort((pp, ww))
            ww, pp, gg, dm = ww[order], pp[order], gg[order], dm[order]
            cnt[d] = np.bincount(ww * 2 + pp, minlength=NW * 2).reshape(NW, 2)
            keys.append((ww, pp, gg, dm))
        nbw = -(-cnt.max(axis=0) // 128)              # [NW, 2] ceil
        zero_w = nbw.sum(axis=1) == 0
        nbw[zero_w, 0] = 1                            # >=1 block per window
        nblk = int(nbw.sum())
        nck = -(-(nblk * 128) // CHUNK)
        nchunk[r] = nck
        nslot_pad = nck * CHUNK

        bm = []
        seg_start = np.zeros(NW * 2, np.int64)
        acc_slots = 0
        for wv in range(NW):
            tot = int(nbw[wv, 0] + nbw[wv, 1])
            j = 0
            for p in (0, 1):
                seg_start[wv * 2 + p] = acc_slots
                acc_slots += int(nbw[wv, p]) * 128
                for _ in range(int(nbw[wv, p])):
                    bm.append((wv, p, j == 0, j == tot - 1))
                    j += 1
        blocks_r[r] = bm

        for d in range(NC):
            ww, pp, gg, dm = keys[d]
            grp = ww * 2 + pp
            gcnt = np.bincount(grp, minlength=NW * 2)
            gstart = np.concatenate([[0], np.cumsum(gcnt)])[grp]
            rank = np.arange(len(ww)) - gstart
            slot = seg_start[grp] + rank
            idx16 = np.zeros(nslot_pad, np.int16)
            idx16[slot] = (gg >> 1).astype(np.int16)
            s_arr = np.zeros((128, nslot_pad), NP_FP8)
            blk = slot >> 7
            s_arr[slot & 127, blk * 128 + dm] = 1.0
            lanes = idx16.reshape(-1, 16).T
            idx_rep = np.tile(lanes, (8, 1))
            per_core[d][r] = (idx_rep, s_arr)
    return onorm, inorm, blocks_r, nchunk, coef, per_core


# --------------------------------------------------------------------------- builder
def _build(blocks_r, nchunk):
    import os
    ablate = os.environ.get("K_ABLATE", "")
    nc = bacc.Bacc("TRN2", target_bir_lowering=False, debug=False, num_devices=NC)

    # ---- dram I/O (replicated params)
    feat_fm = nc.dram_tensor("feat_fm", [3, IN_F, NP_], F32, kind="ExternalInput")
    aW = nc.dram_tensor("aW", [IN_F, NREL, HID], F32, kind="ExternalInput")
    convW = nc.dram_tensor("convW", [EMB, 2 * NREL, HID], BF16, kind="ExternalInput")
    crW = nc.dram_tensor("crW", [128, NREL, HID], F32, kind="ExternalInput")
    biasum = nc.dram_tensor("biasum", [128, NLAYERS * 3, HID], F32, kind="ExternalInput")
    lng = nc.dram_tensor("lng", [128, 3, HID], F32, kind="ExternalInput")
    lnb = nc.dram_tensor("lnb", [128, 3, HID], F32, kind="ExternalInput")
    clsW = nc.dram_tensor("clsW", [HID, 3, OUT], F32, kind="ExternalInput")
    clsb = nc.dram_tensor("clsb", [128, 3, OUT], F32, kind="ExternalInput")
    onormf = nc.dram_tensor("onormf", [NREL, 128, NB], F32, kind="ExternalInput")
    # per-core
    inormw = nc.dram_tensor("inormw", [NREL, 128, NW], F32, kind="ExternalInput")
    coefw = nc.dram_tensor("coefw", [NREL, 128, NW], F32, kind="ExternalInput")
    idx_t, s_t = {}, {}
    for r in range(NREL):
        ns = nchunk[r] * CHUNK
        idx_t[r] = nc.dram_tensor(f"idx_{r}", [128, ns // 16], I16, kind="ExternalInput")
        s_t[r] = nc.dram_tensor(f"s_{r}", [128, ns], FP8, kind="ExternalInput")
    out_t = nc.dram_tensor("out", [3, 128, NW, OUT], F32, kind="ExternalOutput")

    # internal dram
    tables = [[nc.dram_tensor(f"table_{r}_{p}", [NP_, HID], BF16) for p in range(2)]
              for r in range(NREL)]
    bounce = [nc.dram_tensor(f"bounce_{t}", [HID, SHP], BF16) for t in range(3)]
    h_all = [nc.dram_tensor(f"hall_{t}", [NC, HID, SHP], BF16) for t in range(3)]

    from contextlib import ExitStack
    with tile.TileContext(nc) as tc, ExitStack() as ctx:
        p = lambda name, bufs, **kw: ctx.enter_context(tc.tile_pool(name=name, bufs=bufs, **kw))
        wts = p("wts", 1); accp = p("accp", 1)
        featp = p("feat", 2); idxp = p("idx", 6)
        edgep = p("edges", 6); sp = p("sp", 6)
        htp = p("ht", 2); tstp = p("tstage", 3)
        ostagep = p("ostage", 2); bstp = p("bst", 2); hfmp = p("hfm", 2)
        evp = p("ev", 3); lnp = p("ln", 8); x2p = p("x2", 1)
        psw = p("psw", 4, space="PSUM"); psm = p("psm", 2, space="PSUM")
        pst = p("pst", 2, space="PSUM")

        # ---- params to sbuf
        ident = wts.tile([128, 128], F32)
        make_identity(nc, ident[:])
        eps_s = wts.tile([128, 1], F32)
        nc.vector.memset(eps_s[:], EPS)
        aW_s = wts.tile([IN_F, NREL, HID], F32)
        nc.sync.dma_start(out=aW_s[:], in_=aW[:, :, :])
        convW_s = wts.tile([EMB, 2 * NREL, HID], BF16)
        nc.sync.dma_start(out=convW_s[:], in_=convW[:, :, :])
        crW_s = wts.tile([128, NREL, HID], F32)
        nc.sync.dma_start(out=crW_s[:], in_=crW[:, :, :])
        biasum_s = wts.tile([128, NLAYERS * 3, HID], F32)
        nc.sync.dma_start(out=biasum_s[:], in_=biasum[:, :, :])
        lng_s = wts.tile([128, 3, HID], F32)
        nc.sync.dma_start(out=lng_s[:], in_=lng[:, :, :])
        lnb_s = wts.tile([128, 3, HID], F32)
        nc.sync.dma_start(out=lnb_s[:], in_=lnb[:, :, :])
        clsW_s = wts.tile([HID, 3, OUT], F32)
        nc.sync.dma_start(out=clsW_s[:], in_=clsW[:, :, :])
        clsb_s = wts.tile([128, 3, OUT], F32)
        nc.sync.dma_start(out=clsb_s[:], in_=clsb[:, :, :])
        onorm_s = wts.tile([128, NREL, NB], F32)
        nc.sync.dma_start(out=onorm_s[:], in_=onormf[:, :, :].rearrange("r p b -> p r b"))
        inorm_s = wts.tile([128, NREL, NW], F32)
        nc.sync.dma_start(out=inorm_s[:], in_=inormw[:, :, :].rearrange("r p w -> p r w"))
        coef_s = wts.tile([128, NREL, NW], F32)
        nc.sync.dma_start(out=coef_s[:], in_=coefw[:, :, :].rearrange("r p w -> p r w"))

        acc = [accp.tile([128, NW, HID], F32, name=f"acc{t}") for t in range(3)]

        # ---- deferred table-build task queue --------------------------------
        # Tasks are fine-grained closures emitted into the gather instruction
        # stream so the (in-order) PE/DVE/DMA queues never park on a CC that
        # hasn't finished: needs_cc tasks are only popped once the current dst
        # type's gather stream is half done.
        pending = []  # (tag, needs_cc, fn)

        def drain_tag(tag):
            i = 0
            while i < len(pending):
                if pending[i][0] == tag:
                    pending.pop(i)[2]()
                else:
                    i += 1

        def pop_task(progress_ok):
            if pending and (progress_ok or not pending[0][1]):
                pending.pop(0)[2]()
                return True
            return False

        # ---- layer-1 table tasks straight from features (grouped by src type)
        def push_tables_l1(s):
            rels = RELS_OF_S[s]

            def task(g, s=s, rels=rels):
                for cb in range(g * NW, (g + 1) * NW, 7):
                    fc = featp.tile([IN_F, 7 * 128], F32, tag="fc")
                    nc.scalar.dma_start(out=fc[:],
                                        in_=feat_fm[s, :, cb * 128:(cb + 7) * 128])
                    stgs = {}
                    for r in rels:
                        stgs[r] = tstp.tile([128, 7, HID], BF16, tag="tstage",
                                            name=f"stg{r}")
                    for j in range(7):
                        gb = cb + j
                        for r in rels:
                            pt = psm.tile([128, HID], F32, tag="pmm", name="pt")
                            nc.tensor.matmul(pt[:], fc[:, j * 128:(j + 1) * 128],
                                             aW_s[:, r, :], start=True, stop=True)
                            nc.vector.tensor_scalar_mul(stgs[r][:, j, :], pt[:],
                                                        onorm_s[:, r, gb:gb + 1])
                    for r in rels:
                        nc.scalar.dma_start(
                            out=tables[r][1][cb * 128:(cb + 7) * 128, :]
                                .rearrange("(a p) f -> p a f", p=128),
                            in_=stgs[r][:, :, :])

            for g in range(NC):
                pending.append(((1, s), False, lambda g=g: task(g)))

        # ---- table tasks for layers 2,3 from h_all[s] (one task per src rank)
        def push_tables(l, s):
            lp = l & 1
            rels = RELS_OF_S[s]

            def task(r8, l=l, s=s, lp=lp, rels=rels):
                ht = htp.tile([HID, SHP], BF16, tag="ht", name="ht")
                nc.scalar.dma_start(out=ht[:], in_=h_all[s][r8, :, :])
                for cb in range(0, NW, 7):
                    stgs = {}
                    for r in rels:
                        stgs[r] = tstp.tile([128, 7, HID], BF16, tag="tstage",
                                            name=f"stg{r}")
                    for j in range(7):
                        c2 = cb + j
                        gb = r8 * NW + c2
                        for r in rels:
                            pt = psm.tile([128, HID], F32, tag="pmm", name="pt")
                            nc.tensor.matmul(pt[:], ht[:, c2 * 128:(c2 + 1) * 128],
                                             convW_s[:, (l - 2) * NREL + r, :],
                                             start=True, stop=True)
                            nc.vector.tensor_scalar_mul(stgs[r][:, j, :], pt[:],
                                                        onorm_s[:, r, gb:gb + 1])
                    for r in rels:
                        base = r8 * SHP + cb * 128
                        nc.scalar.dma_start(
                            out=tables[r][lp][base:base + 7 * 128, :]
                                .rearrange("(a p) f -> p a f", p=128),
                            in_=stgs[r][:, :, :])

            for r8 in range(NC):
                pending.append(((l, s), True, lambda r8=r8: task(r8)))

        # ---- gather + segment-sum one relation into acc[t]
        def gather_reduce(l, r, t, first, tstate):
            lp = l & 1
            bm = blocks_r[r]
            src_ap = tables[r][lp][:, :].rearrange("(a b) f -> a (b f)", b=2)
            cur = -1
            eb = sb = None
            pwin = None
            nwin = 0
            for blk, (wv, par, isf, isl) in enumerate(bm):
                c = blk // BPC
                if c != cur:
                    cur = c
                    ib = idxp.tile([128, CHUNK // 16], I16, tag="idx")
                    nc.sync.dma_start(
                        out=ib[:],
                        in_=idx_t[r][:, c * (CHUNK // 16):(c + 1) * (CHUNK // 16)])
                    eb = edgep.tile([128, BPC, 128], BF16, tag="eb")
                    if ablate != "nogather":
                        nc.gpsimd.dma_gather(eb[:], src_ap, ib[:], CHUNK, CHUNK, 128)
                    else:
                        nc.vector.memset(eb[:, 0, 0:2], 0.0)
                    sb = sp.tile([128, CHUNK], FP8, tag="sb")
                    if ablate != "nos":
                        nc.sync.dma_start(out=sb[:],
                                          in_=s_t[r][:, c * CHUNK:(c + 1) * CHUNK])
                    else:
                        nc.vector.memset(sb[:, 0:2], 0.0)
                bb = blk % BPC
                if isf:
                    pwin = psw.tile([128, HID], F32, tag="pw")
                nc.tensor.matmul(pwin[:], sb[:, bb * 128:(bb + 1) * 128],
                                 eb[:, bb, par * HID:(par + 1) * HID],
                                 start=isf, stop=isl)
                tstate[0] += 1
                if isl:
                    if first:
                        nc.vector.tensor_scalar_mul(acc[t][:, wv, :], pwin[:],
                                                    inorm_s[:, r, wv:wv + 1])
                    else:
                        tmp = evp.tile([128, HID], F32, tag="ev")
                        nc.vector.tensor_scalar_mul(tmp[:], pwin[:],
                                                    inorm_s[:, r, wv:wv + 1])
                        nc.vector.tensor_tensor(out=acc[t][:, wv, :],
                                                in0=acc[t][:, wv, :], in1=tmp[:],
                                                op=mybir.AluOpType.add)
                    nwin += 1
                    if nwin % 3 == 0:
                        pop_task(tstate[0] >= tstate[1] // 2)

        # ---- emb_b rank-1 correction (layer 1): acc[t] += coef_r (x) crW_r
        def add_emb_bias(t):
            for r in RELS_OF_T[t]:
                tmp3 = x2p.tile([128, NW, HID], F32, tag="x2", name="bias3")
                nc.vector.tensor_tensor(
                    out=tmp3[:],
                    in0=coef_s[:, r, :].rearrange("p (w o) -> p w o", o=1)
                        .to_broadcast([128, NW, HID]),
                    in1=crW_s[:, r, :].rearrange("p (o f) -> p o f", o=1)
                        .to_broadcast([128, NW, HID]),
                    op=mybir.AluOpType.mult)
                nc.vector.tensor_tensor(out=acc[t][:], in0=acc[t][:], in1=tmp3[:],
                                        op=mybir.AluOpType.add)

        # ---- LayerNorm + ReLU in place on acc[t]
        def layer_norm(l, t):
            a = acc[t]
            li = l - 1
            nc.vector.tensor_tensor(
                out=a[:], in0=a[:],
                in1=biasum_s[:, li * 3 + t:li * 3 + t + 1, :].to_broadcast([128, NW, HID]),
                op=mybir.AluOpType.add)
            ssum = lnp.tile([128, NW], F32, tag="ssum")
            nc.vector.tensor_reduce(out=ssum[:], in_=a[:],
                                    axis=mybir.AxisListType.X, op=mybir.AluOpType.add)
            x2 = x2p.tile([128, NW, HID], F32, tag="x2", name="x2t")
            nc.vector.tensor_tensor(out=x2[:], in0=a[:], in1=a[:],
                                    op=mybir.AluOpType.mult)
            s2 = lnp.tile([128, NW], F32, tag="s2")
            nc.vector.tensor_reduce(out=s2[:], in_=x2[:],
                                    axis=mybir.AxisListType.X, op=mybir.AluOpType.add)
            m = lnp.tile([128, NW], F32, tag="m")
            nc.vector.tensor_scalar_mul(m[:], ssum[:], 1.0 / HID)
            msq = lnp.tile([128, NW], F32, tag="msq")
            nc.vector.tensor_tensor(out=msq[:], in0=m[:], in1=m[:],
                                    op=mybir.AluOpType.mult)
            v = lnp.tile([128, NW], F32, tag="v")
            nc.vector.tensor_scalar_mul(v[:], s2[:], 1.0 / HID)
            nc.vector.tensor_tensor(out=v[:], in0=v[:], in1=msq[:],
                                    op=mybir.AluOpType.subtract)
            sd = lnp.tile([128, NW], F32, tag="sd")
            nc.scalar.activation(sd[:], v[:], mybir.ActivationFunctionType.Sqrt,
                                 bias=eps_s[:])
            inv = lnp.tile([128, NW], F32, tag="inv")
            nc.vector.reciprocal(inv[:], sd[:])
            nc.vector.tensor_tensor(out=a[:], in0=a[:],
                                    in1=m[:].rearrange("p (w o) -> p w o", o=1)
                                    .to_broadcast([128, NW, HID]),
                                    op=mybir.AluOpType.subtract)
            nc.vector.tensor_tensor(out=a[:], in0=a[:],
                                    in1=inv[:].rearrange("p (w o) -> p w o", o=1)
                                    .to_broadcast([128, NW, HID]),
                                    op=mybir.AluOpType.mult)
            nc.vector.tensor_tensor(out=a[:], in0=a[:],
                                    in1=lng_s[:, t:t + 1, :].to_broadcast([128, NW, HID]),
                                    op=mybir.AluOpType.mult)
            nc.vector.tensor_tensor(out=a[:], in0=a[:],
                                    in1=lnb_s[:, t:t + 1, :].to_broadcast([128, NW, HID]),
                                    op=mybir.AluOpType.add)
            nc.vector.tensor_scalar_max(a[:], a[:], 0.0)

        # ---- bounce h[t] (feature-major bf16) -> DRAM -> AllGather
        def bounce_cc(t):
            bstage = bstp.tile([HID, SHP], BF16, tag="bst")
            for c in range(NW):
                ptr = pst.tile([HID, 128], F32, tag="ptr")
                nc.tensor.transpose(out=ptr[:], in_=acc[t][:, c, :], identity=ident[:])
                nc.vector.tensor_copy(out=bstage[:, c * 128:(c + 1) * 128], in_=ptr[:])
            nc.scalar.dma_start(out=bounce[t][:, :], in_=bstage[:])
            if ablate == "nocc":
                return
            nc.gpsimd.collective_compute(
                "AllGather", mybir.AluOpType.bypass,
                replica_groups=[list(range(NC))],
                ins=[bounce[t][:, :].opt()], outs=[h_all[t][:, :, :].opt()])

        # ---- classifier for type t from acc[t]
        def classifier(t):
            ostg = ostagep.tile([128, NW, OUT], F32, tag="ostage")
            for c in range(NW):
                ptr = pst.tile([HID, 128], F32, tag="ptr")
                nc.tensor.transpose(out=ptr[:], in_=acc[t][:, c, :], identity=ident[:])
                hfm = hfmp.tile([HID, 128], F32, tag="hfm")
                nc.vector.tensor_copy(out=hfm[:], in_=ptr[:])
                po = psm.tile([128, OUT], F32, tag="pmm", name="po")
                nc.tensor.matmul(po[:], hfm[:], clsW_s[:, t, :], start=True, stop=True)
                nc.vector.tensor_tensor(out=ostg[:, c, :], in0=po[:],
                                        in1=clsb_s[:, t, :], op=mybir.AluOpType.add)
            nc.sync.dma_start(out=out_t[t, :, :, :], in_=ostg[:])

        # ================= program =================
        if ablate != "notab":
            for s in BUILD_ORDER[1]:
                push_tables_l1(s)
        for l in (1, 2, 3):
            for t in T_ORDER[l]:
                rels = sorted(RELS_OF_T[t], key=lambda r: BUILD_ORDER[l].index(REL[r][0]))
                tot = sum(len(blocks_r[r]) for r in rels)
                tstate = [0, tot]
                for i, r in enumerate(rels):
                    drain_tag((l, REL[r][0]))
                    gather_reduce(l, r, t, first=(i == 0), tstate=tstate)
                if l == 1:
                    add_emb_bias(t)
                layer_norm(l, t)
                if l < 3:
                    bounce_cc(t)
                    if ablate != "notab":
                        push_tables(l + 1, t)
                else:
                    classifier(t)
        assert not pending, f"undrained tasks: {[p[0] for p in pending]}"
    nc.compile()
    return nc


# --------------------------------------------------------------------------- entry
def kernel(feat_assmpt, feat_rule, feat_non_assmpt, emb_W, emb_b, conv_W, conv_b,
           ln_g, ln_b, cls_W, cls_b, src, dst):
    feats = [np.asarray(feat_assmpt), np.asarray(feat_rule), np.asarray(feat_non_assmpt)]
    src = np.asarray(src)
    dst = np.asarray(dst)
    emb_W = np.asarray(emb_W, np.float64)
    emb_b = np.asarray(emb_b, np.float64)
    conv_W = np.asarray(conv_W, np.float32)
    conv_b = np.asarray(conv_b, np.float32)

    onorm, inorm, blocks_r, nchunk, coef, per_core = _plan(src, dst)

    # layer-1 folded weights: A_r = embW[s] @ W_1r ; c_r = emb_b[s] @ W_1r
    aW_h = np.zeros((IN_F, NREL, HID), np.float32)
    crW_h = np.zeros((128, NREL, HID), np.float32)
    for r, (s, _) in enumerate(REL):
        aW_h[:, r, :] = (emb_W[s] @ conv_W[0, r].astype(np.float64)).astype(np.float32)
        crW_h[:, r, :] = (emb_b[s] @ conv_W[0, r].astype(np.float64)).astype(np.float32)[None, :]

    # conv bias sums per (l, t)
    biasum = np.zeros((NLAYERS, 3, HID), np.float32)
    for l in range(NLAYERS):
        for r, (_, t) in enumerate(REL):
            biasum[l, t] += conv_b[l, r]

    # feature-major features in padded-global order
    feat_fm = np.zeros((3, IN_F, NP_), np.float32)
    for t in range(3):
        ft = feats[t]  # [N, IN_F]
        for d in range(NC):
            feat_fm[t, :, d * SHP:d * SHP + SH] = ft[d * SH:(d + 1) * SH].T

    # onorm in padded-global block layout [NREL, 128, NB]
    onormf = np.ones((NREL, 128, NB), np.float32)
    gidx = np.arange(NP_)
    rank = gidx // SHP
    loc = gidx - rank * SHP
    valid = loc < SH
    nid = rank * SH + loc
    for r in range(NREL):
        buf = np.ones(NP_, np.float32)
        buf[valid] = onorm[r][nid[valid]]
        onormf[r] = buf.reshape(NB, 128).T

    convW_h = np.ascontiguousarray(
        conv_W[1:3].transpose(2, 0, 1, 3).reshape(EMB, 2 * NREL, HID)
    ).astype(NP_BF16)

    nc = _build(blocks_r, nchunk)

    in_maps = []
    for d in range(NC):
        sl = slice(d * SH, (d + 1) * SH)
        iw = np.ones((NREL, 128, NW), np.float32)
        cw = np.zeros((NREL, 128, NW), np.float32)
        for r in range(NREL):
            pad = np.ones(SHP, np.float32)
            pad[:SH] = inorm[r, sl]
            iw[r] = pad.reshape(NW, 128).T
            pad2 = np.zeros(SHP, np.float32)
            pad2[:SH] = coef[d, r]
            cw[r] = pad2.reshape(NW, 128).T
        m = {
            "feat_fm": feat_fm,
            "aW": aW_h,
            "convW": convW_h,
            "crW": crW_h,
            "biasum": np.broadcast_to(biasum.reshape(NLAYERS * 3, HID),
                                      (128, NLAYERS * 3, HID)).copy(),
            "lng": np.broadcast_to(np.asarray(ln_g, np.float32), (128, 3, HID)).copy(),
            "lnb": np.broadcast_to(np.asarray(ln_b, np.float32), (128, 3, HID)).copy(),
            "clsW": np.ascontiguousarray(np.asarray(cls_W, np.float32).transpose(1, 0, 2)),
            "clsb": np.broadcast_to(np.asarray(cls_b, np.float32), (128, 3, OUT)).copy(),
            "onormf": onormf,
            "inormw": iw,
            "coefw": cw,
        }
        for r in range(NREL):
            idx_rep, s_arr = per_core[d][r]
            m[f"idx_{r}"] = idx_rep
            m[f"s_{r}"] = s_arr
        in_maps.append(m)

    import jax
    fn, in_names, out_names, out_avals, sharding = _make_runner(nc, NC)
    concat_in = [np.concatenate([m[n] for m in in_maps], axis=0) for n in in_names]
    zeros = [np.zeros((NC * a.shape[0], *a.shape[1:]), a.dtype) for a in out_avals]
    dev_in = [jax.device_put(x, sharding) for x in concat_in]
    dev_zeros = [jax.device_put(z, sharding) for z in zeros]
    outs = fn(*dev_in, *dev_zeros)
    res = np.asarray(outs[0]).reshape(NC, 3, 128, NW, OUT)

    import time as _time

    def _timed(n=6):
        ts = []
        for _ in range(n):
            t0 = _time.perf_counter()
            r = fn(*dev_in, *dev_zeros)
            jax.block_until_ready(r)
            ts.append(_time.perf_counter() - t0)
        return ts
    globals()["_last_run"] = _timed

    def _timed_marginal(r1=4, r2=20, reps=3):
        """Pipelined marginal per-call time: launch r back-to-back, block once.
        (w(r2)-w(r1))/(r2-r1) cancels the fixed dispatch latency."""
        def batch(rr):
            t0 = _time.perf_counter()
            for _ in range(rr):
                r = fn(*dev_in, *dev_zeros)
            jax.block_until_ready(r)
            return _time.perf_counter() - t0
        batch(2)  # warm
        ests = []
        for _ in range(reps):
            w1 = batch(r1)
            w2 = batch(r2)
            ests.append((w2 - w1) / (r2 - r1))
        return ests
    globals()["_marginal_run"] = _timed_marginal

    full = np.zeros((3, N, OUT), np.float32)
    for d in range(NC):
        o = res[d]  # [3, 128, NW, OUT]
        for t in range(3):
            rows = o[t].transpose(1, 0, 2).reshape(SHP, OUT)[:SH]
            full[t, d * SH:(d + 1) * SH] = rows
    return full


# revision 13
# speedup vs baseline: 1.6531x; 1.6531x over previous
"""Trainium2 Bass kernel for nn_GCNLearnableModel (3-type heterograph GCN, 9 relations,
3 layers) on 8 NeuronCores.

v2 strategy (graph/data parallel, one SPMD NEFF):
 - Nodes of each type sharded 8 ways (6250/core, padded 6272). Each core owns the
   incoming edges of its dst shard.
 - Per layer: ONE AllGather per node type of the feature-major hidden state
   (bounce [64,6272] bf16 -> h_all [8,64,6272]); each core then builds ALL
   relation message tables locally (T_r = onorm_r * (h[src] @ W_lr), bf16,
   node-major, padded-global row ids). Layer-1 tables are folded straight from
   the input features (A_r = embW @ W_1r precomputed on host) -- no embed pass,
   no CC before layer 1. emb_b's contribution is a host-precomputed rank-1
   term added to acc.
 - Gather-aggregate per (layer, relation): edges sorted by (dst window, src
   parity) so each 128-edge block is parity-homogeneous: dma_gather pair-rows
   (256B) with int16 idx = g>>1, one fp8 one-hot S matmul per block
   (rhs = even or odd half of the gathered pair), PSUM-accumulated per window,
   scaled by in_norm into SBUF acc.
 - LayerNorm+ReLU in place on acc; per-type classifier at the end.
"""
import numpy as np
import ml_dtypes

import concourse.bass as bass
import concourse.bacc as bacc
import concourse.mybir as mybir
import concourse.tile as tile
from concourse.bass_utils import run_bass_kernel_spmd
from concourse.masks import make_identity


def _make_runner(nc, n_cores):
    """jit-once PJRT runner with reusable device inputs (mirrors run_bass_via_pjrt)."""
    import jax
    from jax.sharding import Mesh, PartitionSpec
    from jax.experimental.shard_map import shard_map
    from concourse import bass2jax, mybir as mb

    bass2jax.install_neuronx_cc_hook()
    in_names, out_names, out_avals = [], [], []
    pname = nc.partition_id_tensor.name if nc.partition_id_tensor else None
    for alloc in nc.m.functions[0].allocations:
        if not isinstance(alloc, mb.MemoryLocationSet):
            continue
        name = alloc.memorylocations[0].name
        if alloc.kind == "ExternalInput":
            if name != pname:
                in_names.append(name)
        elif alloc.kind == "ExternalOutput":
            out_names.append(name)
            out_avals.append(jax.core.ShapedArray(tuple(alloc.tensor_shape),
                                                  mb.dt.np(alloc.dtype)))
    n_params = len(in_names)
    all_names = in_names + out_names + ([pname] if pname else [])

    def _body(*args):
        operands = list(args)
        if pname:
            operands.append(bass2jax.partition_id_tensor())
        return tuple(bass2jax._bass_exec_p.bind(
            *operands, out_avals=tuple(out_avals), in_names=tuple(all_names),
            out_names=tuple(out_names), lowering_input_output_aliases=(),
            sim_require_finite=True, sim_require_nnan=True, nc=nc))

    devices = jax.devices()[:n_cores]
    mesh = Mesh(np.asarray(devices), ("core",))
    nin = n_params + len(out_names)
    fn = jax.jit(
        shard_map(_body, mesh=mesh, in_specs=(PartitionSpec("core"),) * nin,
                  out_specs=(PartitionSpec("core"),) * len(out_names),
                  check_rep=False),
        keep_unused=True)
    sharding = jax.sharding.NamedSharding(mesh, PartitionSpec("core"))
    return fn, in_names, out_names, out_avals, sharding

# problem constants (hardcoded per harness contract)
REL = [(0, 1), (2, 1), (2, 0), (0, 0), (1, 2), (1, 0), (0, 0), (1, 1), (2, 2)]
N, IN_F, EMB, HID, OUT, NREL, NLAYERS, E = 50000, 128, 64, 64, 8, 9, 3, 800000
EPS = 1e-5
NC = 8                     # cores
SH = N // NC               # shard size 6250
NW = 49                    # windows of 128 nodes per shard (49*128 = 6272)
SHP = NW * 128             # padded shard 6272
NP_ = SHP * NC             # padded global rows 50176
NB = NP_ // 128            # global 128-blocks 392
CHUNK = 1024               # edge slots per dma_gather call (SWDGE ring holds 1024 descs)
BPC = CHUNK // 128         # blocks per chunk (32)

F32, BF16, FP8, I16 = mybir.dt.float32, mybir.dt.bfloat16, mybir.dt.float8e4, mybir.dt.int16
NP_BF16 = np.dtype(mybir.dt.np(BF16))
NP_FP8 = np.dtype(mybir.dt.np(FP8))

RELS_OF_T = [[r for r in range(NREL) if REL[r][1] == t] for t in range(3)]
RELS_OF_S = [[r for r in range(NREL) if REL[r][0] == s] for s in range(3)]
T_ORDER = {l: [(l - 1 + i) % 3 for i in range(3)] for l in (1, 2, 3)}
# order in which src-type tables become available for layer l
BUILD_ORDER = {1: [0, 1, 2], 2: T_ORDER[1], 3: T_ORDER[2]}


# --------------------------------------------------------------------------- host plan
def _plan(src, dst):
    """Edge plan: parity-homogeneous 128-edge blocks per dst window.

    Returns onorm, inorm [NREL,N]; blocks_r[r] = list of (window, parity,
    is_first, is_last) shared across cores; nchunk[r]; per_core[d][r] =
    (idx_rep [128, ns/16] i16, s_arr [128, ns] fp8); coef [NC, NREL, SH]
    (emb_b rank-1 correction, layer 1 only).
    """
    onorm = np.empty((NREL, N), np.float32)
    inorm = np.empty((NREL, N), np.float32)
    for r in range(NREL):
        od = np.bincount(src[r], minlength=N).astype(np.float32)
        idg = np.bincount(dst[r], minlength=N).astype(np.float32)
        onorm[r] = np.maximum(od, 1.0) ** -0.5
        inorm[r] = np.maximum(idg, 1.0) ** -0.5

    per_core = [dict() for _ in range(NC)]
    blocks_r = {}
    nchunk = {}
    coef = np.zeros((NC, NREL, SH), np.float32)
    for r in range(NREL):
        g = (src[r] // SH) * SHP + (src[r] % SH)      # padded global id
        par = g & 1
        dcore = dst[r] // SH
        dloc = dst[r] - dcore * SH
        w = dloc >> 7
        dmod = dloc & 127

        osrc = onorm[r][src[r]]
        dsum = np.bincount(dst[r], weights=osrc, minlength=N).astype(np.float32)
        for d in range(NC):
            sl = slice(d * SH, (d + 1) * SH)
            coef[d, r] = inorm[r, sl] * dsum[sl]

        cnt = np.zeros((NC, NW, 2), np.int64)
        keys = []
        for d in range(NC):
            m = dcore == d
            ww, pp, gg, dm = w[m], par[m], g[m], dmod[m]
            order = np.lexsort((gg, pp, ww))
            ww, pp, gg, dm = ww[order], pp[order], gg[order], dm[order]
            cnt[d] = np.bincount(ww * 2 + pp, minlength=NW * 2).reshape(NW, 2)
            keys.append((ww, pp, gg, dm))
        nbw = -(-cnt.max(axis=0) // 128)              # [NW, 2] ceil
        zero_w = nbw.sum(axis=1) == 0
        nbw[zero_w, 0] = 1                            # >=1 block per window
        nblk = int(nbw.sum())
        nck = -(-(nblk * 128) // CHUNK)
        nchunk[r] = nck
        nslot_pad = nck * CHUNK

        bm = []
        seg_start = np.zeros(NW * 2, np.int64)
        acc_slots = 0
        for wv in range(NW):
            tot = int(nbw[wv, 0] + nbw[wv, 1])
            j = 0
            for p in (0, 1):
                seg_start[wv * 2 + p] = acc_slots
                acc_slots += int(nbw[wv, p]) * 128
                for _ in range(int(nbw[wv, p])):
                    bm.append((wv, p, j == 0, j == tot - 1))
                    j += 1
        blocks_r[r] = bm

        for d in range(NC):
            ww, pp, gg, dm = keys[d]
            grp = ww * 2 + pp
            gcnt = np.bincount(grp, minlength=NW * 2)
            gstart = np.concatenate([[0], np.cumsum(gcnt)])[grp]
            rank = np.arange(len(ww)) - gstart
            slot = seg_start[grp] + rank
            idx16 = np.zeros(nslot_pad, np.int16)
            idx16[slot] = (gg >> 1).astype(np.int16)
            s_arr = np.zeros((128, nslot_pad), NP_FP8)
            blk = slot >> 7
            s_arr[slot & 127, blk * 128 + dm] = 1.0
            lanes = idx16.reshape(-1, 16).T
            idx_rep = np.tile(lanes, (8, 1))
            per_core[d][r] = (idx_rep, s_arr)
    return onorm, inorm, blocks_r, nchunk, coef, per_core


# --------------------------------------------------------------------------- builder
def _build(blocks_r, nchunk):
    import os
    ablate = os.environ.get("K_ABLATE", "")
    nc = bacc.Bacc("TRN2", target_bir_lowering=False, debug=False, num_devices=NC,
                   num_swdge_queues=4)

    # ---- dram I/O (replicated params)
    feat_fm = nc.dram_tensor("feat_fm", [3, IN_F, NP_], F32, kind="ExternalInput")
    aW = nc.dram_tensor("aW", [IN_F, NREL, HID], F32, kind="ExternalInput")
    convW = nc.dram_tensor("convW", [EMB, 2 * NREL, HID], BF16, kind="ExternalInput")
    crW = nc.dram_tensor("crW", [128, NREL, HID], F32, kind="ExternalInput")
    biasum = nc.dram_tensor("biasum", [128, NLAYERS * 3, HID], F32, kind="ExternalInput")
    lng = nc.dram_tensor("lng", [128, 3, HID], F32, kind="ExternalInput")
    lnb = nc.dram_tensor("lnb", [128, 3, HID], F32, kind="ExternalInput")
    clsW = nc.dram_tensor("clsW", [HID, 3, OUT], F32, kind="ExternalInput")
    clsb = nc.dram_tensor("clsb", [128, 3, OUT], F32, kind="ExternalInput")
    onormf = nc.dram_tensor("onormf", [NREL, 128, NB], F32, kind="ExternalInput")
    # per-core
    inormw = nc.dram_tensor("inormw", [NREL, 128, NW], F32, kind="ExternalInput")
    coefw = nc.dram_tensor("coefw", [NREL, 128, NW], F32, kind="ExternalInput")
    idx_t, s_t = {}, {}
    for r in range(NREL):
        ns = nchunk[r] * CHUNK
        idx_t[r] = nc.dram_tensor(f"idx_{r}", [128, ns // 16], I16, kind="ExternalInput")
        s_t[r] = nc.dram_tensor(f"s_{r}", [128, ns], FP8, kind="ExternalInput")
    out_t = nc.dram_tensor("out", [3, 128, NW, OUT], F32, kind="ExternalOutput")

    # internal dram
    tables = [[nc.dram_tensor(f"table_{r}_{p}", [NP_, HID], BF16) for p in range(2)]
              for r in range(NREL)]
    bounce = [nc.dram_tensor(f"bounce_{t}", [HID, SHP], BF16) for t in range(3)]
    h_all = [nc.dram_tensor(f"hall_{t}", [NC, HID, SHP], BF16) for t in range(3)]

    from contextlib import ExitStack
    with tile.TileContext(nc) as tc, ExitStack() as ctx:
        p = lambda name, bufs, **kw: ctx.enter_context(tc.tile_pool(name=name, bufs=bufs, **kw))
        wts = p("wts", 1); accp = p("accp", 1)
        featp = p("feat", 2); idxp = p("idx", 6)
        edgep = p("edges", 6); sp = p("sp", 6)
        htp = p("ht", 2); tstp = p("tstage", 3)
        ostagep = p("ostage", 2); bstp = p("bst", 2); hfmp = p("hfm", 2)
        evp = p("ev", 3); lnp = p("ln", 8); x2p = p("x2", 1)
        psw = p("psw", 4, space="PSUM"); psm = p("psm", 2, space="PSUM")
        pst = p("pst", 2, space="PSUM")

        # ---- params to sbuf
        ident = wts.tile([128, 128], F32)
        make_identity(nc, ident[:])
        eps_s = wts.tile([128, 1], F32)
        nc.vector.memset(eps_s[:], EPS)
        aW_s = wts.tile([IN_F, NREL, HID], F32)
        nc.sync.dma_start(out=aW_s[:], in_=aW[:, :, :])
        convW_s = wts.tile([EMB, 2 * NREL, HID], BF16)
        nc.sync.dma_start(out=convW_s[:], in_=convW[:, :, :])
        crW_s = wts.tile([128, NREL, HID], F32)
        nc.sync.dma_start(out=crW_s[:], in_=crW[:, :, :])
        biasum_s = wts.tile([128, NLAYERS * 3, HID], F32)
        nc.sync.dma_start(out=biasum_s[:], in_=biasum[:, :, :])
        lng_s = wts.tile([128, 3, HID], F32)
        nc.sync.dma_start(out=lng_s[:], in_=lng[:, :, :])
        lnb_s = wts.tile([128, 3, HID], F32)
        nc.sync.dma_start(out=lnb_s[:], in_=lnb[:, :, :])
        clsW_s = wts.tile([HID, 3, OUT], F32)
        nc.sync.dma_start(out=clsW_s[:], in_=clsW[:, :, :])
        clsb_s = wts.tile([128, 3, OUT], F32)
        nc.sync.dma_start(out=clsb_s[:], in_=clsb[:, :, :])
        onorm_s = wts.tile([128, NREL, NB], F32)
        nc.sync.dma_start(out=onorm_s[:], in_=onormf[:, :, :].rearrange("r p b -> p r b"))
        inorm_s = wts.tile([128, NREL, NW], F32)
        nc.sync.dma_start(out=inorm_s[:], in_=inormw[:, :, :].rearrange("r p w -> p r w"))
        coef_s = wts.tile([128, NREL, NW], F32)
        nc.sync.dma_start(out=coef_s[:], in_=coefw[:, :, :].rearrange("r p w -> p r w"))

        acc = [accp.tile([128, NW, HID], F32, name=f"acc{t}") for t in range(3)]

        # ---- deferred table-build task queue --------------------------------
        # Tasks are fine-grained closures emitted into the gather instruction
        # stream so the (in-order) PE/DVE/DMA queues never park on a CC that
        # hasn't finished: needs_cc tasks are only popped once the current dst
        # type's gather stream is half done.
        pending = []  # (tag, needs_cc, fn)

        def drain_tag(tag):
            i = 0
            while i < len(pending):
                if pending[i][0] == tag:
                    pending.pop(i)[2]()
                else:
                    i += 1

        def pop_task(progress_ok):
            if pending and (progress_ok or not pending[0][1]):
                pending.pop(0)[2]()
                return True
            return False

        # ---- layer-1 table tasks straight from features (grouped by src type)
        def push_tables_l1(s):
            rels = RELS_OF_S[s]

            def task(g, s=s, rels=rels):
                for cb in range(g * NW, (g + 1) * NW, 7):
                    fc = featp.tile([IN_F, 7 * 128], F32, tag="fc")
                    nc.scalar.dma_start(out=fc[:],
                                        in_=feat_fm[s, :, cb * 128:(cb + 7) * 128])
                    stgs = {}
                    for r in rels:
                        stgs[r] = tstp.tile([128, 7, HID], BF16, tag="tstage",
                                            name=f"stg{r}")
                    for j in range(7):
                        gb = cb + j
                        for r in rels:
                            pt = psm.tile([128, HID], F32, tag="pmm", name="pt")
                            nc.tensor.matmul(pt[:], fc[:, j * 128:(j + 1) * 128],
                                             aW_s[:, r, :], start=True, stop=True)
                            nc.vector.tensor_scalar_mul(stgs[r][:, j, :], pt[:],
                                                        onorm_s[:, r, gb:gb + 1])
                    for r in rels:
                        nc.scalar.dma_start(
                            out=tables[r][1][cb * 128:(cb + 7) * 128, :]
                                .rearrange("(a p) f -> p a f", p=128),
                            in_=stgs[r][:, :, :])

            for g in range(NC):
                pending.append(((1, s), False, lambda g=g: task(g)))

        # ---- table tasks for layers 2,3 from h_all[s] (one task per src rank)
        def push_tables(l, s):
            lp = l & 1
            rels = RELS_OF_S[s]

            def task(r8, l=l, s=s, lp=lp, rels=rels):
                ht = htp.tile([HID, SHP], BF16, tag="ht", name="ht")
                nc.scalar.dma_start(out=ht[:], in_=h_all[s][r8, :, :])
                for cb in range(0, NW, 7):
                    stgs = {}
                    for r in rels:
                        stgs[r] = tstp.tile([128, 7, HID], BF16, tag="tstage",
                                            name=f"stg{r}")
                    for j in range(7):
                        c2 = cb + j
                        gb = r8 * NW + c2
                        for r in rels:
                            pt = psm.tile([128, HID], F32, tag="pmm", name="pt")
                            nc.tensor.matmul(pt[:], ht[:, c2 * 128:(c2 + 1) * 128],
                                             convW_s[:, (l - 2) * NREL + r, :],
                                             start=True, stop=True)
                            nc.vector.tensor_scalar_mul(stgs[r][:, j, :], pt[:],
                                                        onorm_s[:, r, gb:gb + 1])
                    for r in rels:
                        base = r8 * SHP + cb * 128
                        nc.scalar.dma_start(
                            out=tables[r][lp][base:base + 7 * 128, :]
                                .rearrange("(a p) f -> p a f", p=128),
                            in_=stgs[r][:, :, :])

            for r8 in range(NC):
                pending.append(((l, s), True, lambda r8=r8: task(r8)))

        # ---- gather + segment-sum one relation into acc[t]
        def gather_reduce(l, r, t, first, tstate):
            lp = l & 1
            bm = blocks_r[r]
            src_ap = tables[r][lp][:, :].rearrange("(a b) f -> a (b f)", b=2)
            cur = -1
            eb = sb = None
            pwin = None
            nwin = 0
            for blk, (wv, par, isf, isl) in enumerate(bm):
                c = blk // BPC
                if c != cur:
                    cur = c
                    ib = idxp.tile([128, CHUNK // 16], I16, tag="idx")
                    nc.sync.dma_start(
                        out=ib[:],
                        in_=idx_t[r][:, c * (CHUNK // 16):(c + 1) * (CHUNK // 16)])
                    eb = edgep.tile([128, BPC, 128], BF16, tag="eb")
                    if ablate != "nogather":
                        nc.gpsimd.dma_gather(eb[:], src_ap, ib[:], CHUNK, CHUNK, 128,
                                             queue_num=c % 4)
                    else:
                        nc.vector.memset(eb[:, 0, 0:2], 0.0)
                    sb = sp.tile([128, CHUNK], FP8, tag="sb")
                    if ablate != "nos":
                        nc.sync.dma_start(out=sb[:],
                                          in_=s_t[r][:, c * CHUNK:(c + 1) * CHUNK])
                    else:
                        nc.vector.memset(sb[:, 0:2], 0.0)
                bb = blk % BPC
                if isf:
                    pwin = psw.tile([128, HID], F32, tag="pw")
                nc.tensor.matmul(pwin[:], sb[:, bb * 128:(bb + 1) * 128],
                                 eb[:, bb, par * HID:(par + 1) * HID],
                                 start=isf, stop=isl)
                tstate[0] += 1
                if isl:
                    if first:
                        nc.vector.tensor_scalar_mul(acc[t][:, wv, :], pwin[:],
                                                    inorm_s[:, r, wv:wv + 1])
                    else:
                        tmp = evp.tile([128, HID], F32, tag="ev")
                        nc.vector.tensor_scalar_mul(tmp[:], pwin[:],
                                                    inorm_s[:, r, wv:wv + 1])
                        nc.vector.tensor_tensor(out=acc[t][:, wv, :],
                                                in0=acc[t][:, wv, :], in1=tmp[:],
                                                op=mybir.AluOpType.add)
                    nwin += 1
                    if nwin % 3 == 0:
                        pop_task(tstate[0] >= tstate[1] // 2)

        # ---- emb_b rank-1 correction (layer 1): acc[t] += coef_r (x) crW_r
        def add_emb_bias(t):
            for r in RELS_OF_T[t]:
                tmp3 = x2p.tile([128, NW, HID], F32, tag="x2", name="bias3")
                nc.vector.tensor_tensor(
                    out=tmp3[:],
                    in0=coef_s[:, r, :].rearrange("p (w o) -> p w o", o=1)
                        .to_broadcast([128, NW, HID]),
                    in1=crW_s[:, r, :].rearrange("p (o f) -> p o f", o=1)
                        .to_broadcast([128, NW, HID]),
                    op=mybir.AluOpType.mult)
                nc.vector.tensor_tensor(out=acc[t][:], in0=acc[t][:], in1=tmp3[:],
                                        op=mybir.AluOpType.add)

        # ---- LayerNorm + ReLU in place on acc[t]
        def layer_norm(l, t):
            a = acc[t]
            li = l - 1
            nc.vector.tensor_tensor(
                out=a[:], in0=a[:],
                in1=biasum_s[:, li * 3 + t:li * 3 + t + 1, :].to_broadcast([128, NW, HID]),
                op=mybir.AluOpType.add)
            ssum = lnp.tile([128, NW], F32, tag="ssum")
            nc.vector.tensor_reduce(out=ssum[:], in_=a[:],
                                    axis=mybir.AxisListType.X, op=mybir.AluOpType.add)
            x2 = x2p.tile([128, NW, HID], F32, tag="x2", name="x2t")
            nc.vector.tensor_tensor(out=x2[:], in0=a[:], in1=a[:],
                                    op=mybir.AluOpType.mult)
            s2 = lnp.tile([128, NW], F32, tag="s2")
            nc.vector.tensor_reduce(out=s2[:], in_=x2[:],
                                    axis=mybir.AxisListType.X, op=mybir.AluOpType.add)
            m = lnp.tile([128, NW], F32, tag="m")
            nc.vector.tensor_scalar_mul(m[:], ssum[:], 1.0 / HID)
            msq = lnp.tile([128, NW], F32, tag="msq")
            nc.vector.tensor_tensor(out=msq[:], in0=m[:], in1=m[:],
                                    op=mybir.AluOpType.mult)
            v = lnp.tile([128, NW], F32, tag="v")
            nc.vector.tensor_scalar_mul(v[:], s2[:], 1.0 / HID)
            nc.vector.tensor_tensor(out=v[:], in0=v[:], in1=msq[:],
                                    op=mybir.AluOpType.subtract)
            sd = lnp.tile([128, NW], F32, tag="sd")
            nc.scalar.activation(sd[:], v[:], mybir.ActivationFunctionType.Sqrt,
                                 bias=eps_s[:])
            inv = lnp.tile([128, NW], F32, tag="inv")
            nc.vector.reciprocal(inv[:], sd[:])
            nc.vector.tensor_tensor(out=a[:], in0=a[:],
                                    in1=m[:].rearrange("p (w o) -> p w o", o=1)
                                    .to_broadcast([128, NW, HID]),
                                    op=mybir.AluOpType.subtract)
            nc.vector.tensor_tensor(out=a[:], in0=a[:],
                                    in1=inv[:].rearrange("p (w o) -> p w o", o=1)
                                    .to_broadcast([128, NW, HID]),
                                    op=mybir.AluOpType.mult)
            nc.vector.tensor_tensor(out=a[:], in0=a[:],
                                    in1=lng_s[:, t:t + 1, :].to_broadcast([128, NW, HID]),
                                    op=mybir.AluOpType.mult)
            nc.vector.tensor_tensor(out=a[:], in0=a[:],
                                    in1=lnb_s[:, t:t + 1, :].to_broadcast([128, NW, HID]),
                                    op=mybir.AluOpType.add)
            nc.vector.tensor_scalar_max(a[:], a[:], 0.0)

        # ---- bounce h[t] (feature-major bf16) -> DRAM -> AllGather
        def bounce_cc(t):
            bstage = bstp.tile([HID, SHP], BF16, tag="bst")
            for c in range(NW):
                ptr = pst.tile([HID, 128], F32, tag="ptr")
                nc.tensor.transpose(out=ptr[:], in_=acc[t][:, c, :], identity=ident[:])
                nc.vector.tensor_copy(out=bstage[:, c * 128:(c + 1) * 128], in_=ptr[:])
            nc.scalar.dma_start(out=bounce[t][:, :], in_=bstage[:])
            if ablate == "nocc":
                return
            nc.gpsimd.collective_compute(
                "AllGather", mybir.AluOpType.bypass,
                replica_groups=[list(range(NC))],
                ins=[bounce[t][:, :].opt()], outs=[h_all[t][:, :, :].opt()])

        # ---- classifier for type t from acc[t]
        def classifier(t):
            ostg = ostagep.tile([128, NW, OUT], F32, tag="ostage")
            for c in range(NW):
                ptr = pst.tile([HID, 128], F32, tag="ptr")
                nc.tensor.transpose(out=ptr[:], in_=acc[t][:, c, :], identity=ident[:])
                hfm = hfmp.tile([HID, 128], F32, tag="hfm")
                nc.vector.tensor_copy(out=hfm[:], in_=ptr[:])
                po = psm.tile([128, OUT], F32, tag="pmm", name="po")
                nc.tensor.matmul(po[:], hfm[:], clsW_s[:, t, :], start=True, stop=True)
                nc.vector.tensor_tensor(out=ostg[:, c, :], in0=po[:],
                                        in1=clsb_s[:, t, :], op=mybir.AluOpType.add)
            nc.sync.dma_start(out=out_t[t, :, :, :], in_=ostg[:])

        # ================= program =================
        if ablate != "notab":
            for s in BUILD_ORDER[1]:
                push_tables_l1(s)
        for l in (1, 2, 3):
            for t in T_ORDER[l]:
                rels = sorted(RELS_OF_T[t], key=lambda r: BUILD_ORDER[l].index(REL[r][0]))
                tot = sum(len(blocks_r[r]) for r in rels)
                tstate = [0, tot]
                for i, r in enumerate(rels):
                    drain_tag((l, REL[r][0]))
                    gather_reduce(l, r, t, first=(i == 0), tstate=tstate)
                if l == 1:
                    add_emb_bias(t)
                layer_norm(l, t)
                if l < 3:
                    bounce_cc(t)
                    if ablate != "notab":
                        push_tables(l + 1, t)
                else:
                    classifier(t)
        assert not pending, f"undrained tasks: {[p[0] for p in pending]}"
    nc.compile()
    return nc


# --------------------------------------------------------------------------- entry
def kernel(feat_assmpt, feat_rule, feat_non_assmpt, emb_W, emb_b, conv_W, conv_b,
           ln_g, ln_b, cls_W, cls_b, src, dst):
    feats = [np.asarray(feat_assmpt), np.asarray(feat_rule), np.asarray(feat_non_assmpt)]
    src = np.asarray(src)
    dst = np.asarray(dst)
    emb_W = np.asarray(emb_W, np.float64)
    emb_b = np.asarray(emb_b, np.float64)
    conv_W = np.asarray(conv_W, np.float32)
    conv_b = np.asarray(conv_b, np.float32)

    onorm, inorm, blocks_r, nchunk, coef, per_core = _plan(src, dst)

    # layer-1 folded weights: A_r = embW[s] @ W_1r ; c_r = emb_b[s] @ W_1r
    aW_h = np.zeros((IN_F, NREL, HID), np.float32)
    crW_h = np.zeros((128, NREL, HID), np.float32)
    for r, (s, _) in enumerate(REL):
        aW_h[:, r, :] = (emb_W[s] @ conv_W[0, r].astype(np.float64)).astype(np.float32)
        crW_h[:, r, :] = (emb_b[s] @ conv_W[0, r].astype(np.float64)).astype(np.float32)[None, :]

    # conv bias sums per (l, t)
    biasum = np.zeros((NLAYERS, 3, HID), np.float32)
    for l in range(NLAYERS):
        for r, (_, t) in enumerate(REL):
            biasum[l, t] += conv_b[l, r]

    # feature-major features in padded-global order
    feat_fm = np.zeros((3, IN_F, NP_), np.float32)
    for t in range(3):
        ft = feats[t]  # [N, IN_F]
        for d in range(NC):
            feat_fm[t, :, d * SHP:d * SHP + SH] = ft[d * SH:(d + 1) * SH].T

    # onorm in padded-global block layout [NREL, 128, NB]
    onormf = np.ones((NREL, 128, NB), np.float32)
    gidx = np.arange(NP_)
    rank = gidx // SHP
    loc = gidx - rank * SHP
    valid = loc < SH
    nid = rank * SH + loc
    for r in range(NREL):
        buf = np.ones(NP_, np.float32)
        buf[valid] = onorm[r][nid[valid]]
        onormf[r] = buf.reshape(NB, 128).T

    convW_h = np.ascontiguousarray(
        conv_W[1:3].transpose(2, 0, 1, 3).reshape(EMB, 2 * NREL, HID)
    ).astype(NP_BF16)

    nc = _build(blocks_r, nchunk)

    in_maps = []
    for d in range(NC):
        sl = slice(d * SH, (d + 1) * SH)
        iw = np.ones((NREL, 128, NW), np.float32)
        cw = np.zeros((NREL, 128, NW), np.float32)
        for r in range(NREL):
            pad = np.ones(SHP, np.float32)
            pad[:SH] = inorm[r, sl]
            iw[r] = pad.reshape(NW, 128).T
            pad2 = np.zeros(SHP, np.float32)
            pad2[:SH] = coef[d, r]
            cw[r] = pad2.reshape(NW, 128).T
        m = {
            "feat_fm": feat_fm,
            "aW": aW_h,
            "convW": convW_h,
            "crW": crW_h,
            "biasum": np.broadcast_to(biasum.reshape(NLAYERS * 3, HID),
                                      (128, NLAYERS * 3, HID)).copy(),
            "lng": np.broadcast_to(np.asarray(ln_g, np.float32), (128, 3, HID)).copy(),
            "lnb": np.broadcast_to(np.asarray(ln_b, np.float32), (128, 3, HID)).copy(),
            "clsW": np.ascontiguousarray(np.asarray(cls_W, np.float32).transpose(1, 0, 2)),
            "clsb": np.broadcast_to(np.asarray(cls_b, np.float32), (128, 3, OUT)).copy(),
            "onormf": onormf,
            "inormw": iw,
            "coefw": cw,
        }
        for r in range(NREL):
            idx_rep, s_arr = per_core[d][r]
            m[f"idx_{r}"] = idx_rep
            m[f"s_{r}"] = s_arr
        in_maps.append(m)

    import jax
    fn, in_names, out_names, out_avals, sharding = _make_runner(nc, NC)
    concat_in = [np.concatenate([m[n] for m in in_maps], axis=0) for n in in_names]
    zeros = [np.zeros((NC * a.shape[0], *a.shape[1:]), a.dtype) for a in out_avals]
    dev_in = [jax.device_put(x, sharding) for x in concat_in]
    dev_zeros = [jax.device_put(z, sharding) for z in zeros]
    outs = fn(*dev_in, *dev_zeros)
    res = np.asarray(outs[0]).reshape(NC, 3, 128, NW, OUT)

    import time as _time

    def _timed(n=6):
        ts = []
        for _ in range(n):
            t0 = _time.perf_counter()
            r = fn(*dev_in, *dev_zeros)
            jax.block_until_ready(r)
            ts.append(_time.perf_counter() - t0)
        return ts
    globals()["_last_run"] = _timed

    def _timed_marginal(r1=4, r2=20, reps=3):
        """Pipelined marginal per-call time: launch r back-to-back, block once.
        (w(r2)-w(r1))/(r2-r1) cancels the fixed dispatch latency."""
        def batch(rr):
            t0 = _time.perf_counter()
            for _ in range(rr):
                r = fn(*dev_in, *dev_zeros)
            jax.block_until_ready(r)
            return _time.perf_counter() - t0
        batch(2)  # warm
        ests = []
        for _ in range(reps):
            w1 = batch(r1)
            w2 = batch(r2)
            ests.append((w2 - w1) / (r2 - r1))
        return ests
    globals()["_marginal_run"] = _timed_marginal

    full = np.zeros((3, N, OUT), np.float32)
    for d in range(NC):
        o = res[d]  # [3, 128, NW, OUT]
        for t in range(3):
            rows = o[t].transpose(1, 0, 2).reshape(SHP, OUT)[:SH]
            full[t, d * SH:(d + 1) * SH] = rows
    return full
